# revision 16
# baseline (speedup 1.0000x reference)
"""Trainium2 Bass kernel for nn_CausalRankKAttention.

Blend of banded-softmax attention and cumsum linear attention, per (n,h) pair.
16 pairs sharded over 8 NeuronCores (2 pairs/core), no cross-core comm.

Design (v3):
  - feature map phi(x)=tanh(x)+1 on HOST; only the exp table ever loads on ACT.
  - q/k for the softmax path in fp8e4m3 with DoubleRow matmuls (2 contraction
    rows/cycle): tail + band score matmuls run at half cycle cost. fp8 error
    feeds only exp logits (~0.05 abs) -> ~0.5% on softmax sums, well under the
    2e-2 gate.
  - per q-block lb: tail = forward scores vs s-blocks [0, lb), one wide psum,
    single ACT exp with fused accum_out -> denominator tail. band = transposed
    tile st[k=lb, q in lb..lb+1] -> exp -> parallelogram mask (Pool) -> one
    "mv" matmul vsm^T @ st_m -> [65, 256] numerators + masses (row 64).
  - linear path per chunk: transposed scores -> causal mask (DVE) -> atv+inter
    into [65, 128] psum; state S [64, 65] from a kn matmul, updated on DVE.
  - softmax blocks processed DESCENDING (big tails first) while linear chunks
    ascend; the dependent matmuls (mv / inter / atv) trail by one iteration so
    the PE never waits on ACT/DVE/Pool; band+linear share one [128, 1024] psum
    tile per iteration and drain with a single [65, 384] CAST.
  - outputs are RAW numerators/denominators; normalize + blend on host.
"""

import numpy as np
import ml_dtypes

import concourse.bass as bass
import concourse.bacc as bacc
import concourse.mybir as mybir
import concourse.tile as tile
from concourse import bass_utils

F32 = mybir.dt.float32
BF16 = mybir.dt.bfloat16
FP8 = mybir.dt.float8e4
DR = mybir.MatmulPerfMode.DoubleRow
AF = mybir.ActivationFunctionType
OP = mybir.AluOpType

N, L, H, E = 2, 2048, 8, 64
NB = L // 128            # 16 blocks/chunks of 128
TEMP = float(1.0 / np.sqrt(E))
EPS = 1e-6
NEGF8 = -448.0           # fp8e4m3 max magnitude; *TEMP -> exp(-56) == 0
PAIRS_PER_CORE = 2
NCORES = 8

_cached = {}


def build_nc():
    nc = bacc.Bacc("TRN2", target_bir_lowering=False, debug=False,
                   num_devices=NCORES)
    P = PAIRS_PER_CORE
    # ---- dram tensors (per core) ----
    # qkt[p, :, 0] = kt (k^T + gate ext row), [p, :, 1] = qt (q^T + ones row)
    qkt = nc.dram_tensor("qkt", [P, 65, 2, L], BF16, kind="ExternalInput")
    sg = nc.dram_tensor("sg", [P, 64, 2, L], BF16, kind="ExternalInput")
    # vvkn: [128, 2*NB*65 vv | NB*64 sgkn]
    vvkn = nc.dram_tensor("vvkn", [P, 128, 2 * NB * 65 + NB * 64], BF16,
                          kind="ExternalInput")
    m01d = nc.dram_tensor("m01d", [128, 256], BF16, kind="ExternalInput")
    svlv = nc.dram_tensor("svlv", [P, 65, NB, 384], BF16, kind="ExternalOutput")
    tails = nc.dram_tensor("tails", [P, 128, NB], F32, kind="ExternalOutput")

    with tile.TileContext(nc) as tc:
        with (
            tc.tile_pool(name="const", bufs=1) as constp,
            tc.tile_pool(name="io", bufs=2) as iop,
            tc.tile_pool(name="acc", bufs=2) as accp,
            tc.tile_pool(name="work", bufs=2) as workp,
            tc.tile_pool(name="sp", bufs=3) as sp,
            tc.tile_pool(name="tailp", bufs=1, space="PSUM") as tailp,
            tc.tile_pool(name="blp", bufs=2, space="PSUM") as blp,
        ):
            m01_sb = constp.tile([128, 256], BF16, tag="m01")
            nc.sync.dma_start(m01_sb[:], m01d[:])

            for p in range(P):
                qkt_sb = iop.tile([65, 2, L], BF16, tag="qkt")
                sg_sb = iop.tile([64, 2, L], BF16, tag="sg")
                vvkn_sb = iop.tile([128, 2 * NB * 65 + NB * 64], BF16, tag="vvkn")
                # DMA priority: kt -> qt blocks 12-15 (first tails) -> sg half 1
                # (chunk-0 scores) -> vlin0 -> vsm -> qt rest -> vlin rest +
                # sgkn -> sg half 2
                nc.sync.dma_start(qkt_sb[:, 0, :], qkt[p, :, 0, :])
                nc.sync.dma_start(qkt_sb[:, 1, 1536:2048], qkt[p, :, 1, 1536:2048])
                nc.sync.dma_start(sg_sb[:, :, 0:1024], sg[p, :, :, 0:1024])
                nc.sync.dma_start(vvkn_sb[:, 0:65], vvkn[p, :, 0:65])
                nc.sync.dma_start(vvkn_sb[:, 65:1105], vvkn[p, :, 65:1105])
                nc.sync.dma_start(qkt_sb[:, 1, 0:1536], qkt[p, :, 1, 0:1536])
                nc.sync.dma_start(vvkn_sb[:, 1105:3104], vvkn[p, :, 1105:3104])
                nc.sync.dma_start(sg_sb[:, :, 1024:2048], sg[p, :, :, 1024:2048])
                kt_sb = qkt_sb[:, 0, :]       # [65, L]
                qt_sb = qkt_sb[:, 1, :]
                sgk_sb = sg_sb[:, 0, :]
                sgq_sb = sg_sb[:, 1, :]

                # vvkn columns: [vlin0 | vsm 0..15 | vlin 1..15 | sgkn]
                def vlin_ap(c):
                    off = 0 if c == 0 else 1105 + (c - 1) * 65
                    return vvkn_sb[:, off:off + 65]

                def vsm_ap(i):
                    return vvkn_sb[:, 65 + i * 65:65 + (i + 1) * 65]

                def sgkn_ap(c):
                    return vvkn_sb[:, 2080 + c * 64:2080 + (c + 1) * 64]

                acc = accp.tile([65, NB, 384], BF16, tag="acc")
                tails_acc = accp.tile([128, NB], F32, tag="tails")
                nc.gpsimd.memset(tails_acc[:], 0.0)

                s_cur = None     # state after chunk n (bf16 [64, 65])
                prev = None
                for n in range(NB):
                    i = NB - 1 - n       # softmax block (descending)
                    c = n                # linear chunk (ascending)
                    qw = 256 if i < NB - 1 else 128
                    c0, c1 = c * 128, (c + 1) * 128
                    b0, b1 = i * 128, (i + 1) * 128

                    # ---- PE: band scores (transposed) ----
                    tl = blp.tile([128, 1024], F32, tag="tl")
                    nc.tensor.matmul(tl[:, 0:qw], kt_sb[:, b0:b1],
                                     qt_sb[:, b0:b0 + qw],
                                     start=True, stop=True)
                    # ---- PE: finish previous tile: mv, inter, atv ----
                    if prev is not None:
                        ptl = prev["tl"]
                        nc.tensor.matmul(ptl[0:65, 512:512 + prev["qw"]],
                                         vsm_ap(prev["i"]),
                                         prev["st_m"][:, 0:prev["qw"]],
                                         start=True, stop=True)
                    # ---- PE: linear qk scores for chunk c ----
                    nc.tensor.matmul(tl[:, 256:384], sgk_sb[:, c0:c1],
                                     sgq_sb[:, c0:c1], start=True, stop=True)
                    if prev is not None:
                        pc = c - 1
                        if pc > 0:
                            nc.tensor.matmul(ptl[0:65, 768:896],
                                             prev["s_before"][:],
                                             sgq_sb[:, pc * 128:pc * 128 + 128],
                                             start=True, stop=False)
                        nc.tensor.matmul(ptl[0:65, 768:896],
                                         vlin_ap(pc), prev["at"][:],
                                         start=(pc == 0), stop=True)
                        nc.vector.tensor_copy(acc[:, n - 1, :],
                                              ptl[0:65, 512:896])
                    # ---- PE: kn (state delta for chunk c) ----
                    nc.tensor.matmul(tl[0:64, 384:449], sgkn_ap(c),
                                     vlin_ap(c), start=True, stop=True)
                    # ---- PE: tail for block i (last: WAR wait on previous
                    # exp overlaps the matmuls above) ----
                    if i >= 1:
                        w = i * 128
                        tp = tailp.tile([128, 2048], F32, tag="tp")
                        for off in range(0, w, 512):
                            n_ = min(512, w - off)
                            nc.tensor.matmul(tp[:, off:off + n_],
                                             qt_sb[:, b0:b1],
                                             kt_sb[:, off:off + n_],
                                             start=True, stop=True)

                    # ---- ACT: band exp, then tail exp ----
                    st_e = workp.tile([128, 256], BF16, tag="st_e")
                    nc.scalar.activation(st_e[:, 0:qw], tl[:, 0:qw], AF.Exp,
                                         scale=TEMP)
                    if i >= 1:
                        scrap = workp.tile([128, 1920], BF16, tag="scrap")
                        nc.scalar.activation(scrap[:, 0:w], tp[:, 0:w], AF.Exp,
                                             scale=TEMP,
                                             accum_out=tails_acc[:, i:i + 1])
                    # ---- DVE: masks + state update ----
                    st_m = workp.tile([128, 256], BF16, tag="st_m")
                    nc.vector.tensor_tensor(st_m[:, 0:qw], st_e[:, 0:qw],
                                            m01_sb[:, 0:qw], OP.mult)
                    at = workp.tile([128, 128], BF16, tag="at")
                    nc.vector.tensor_tensor(at[:], tl[:, 256:384],
                                            m01_sb[:, 0:128], OP.mult)
                    s_before = s_cur
                    s_nxt = sp.tile([64, 65], BF16, tag="s")
                    if n == 0:
                        nc.vector.tensor_copy(s_nxt[:], tl[0:64, 384:449])
                    else:
                        nc.vector.scalar_tensor_tensor(s_nxt[:], s_cur[:], 1.0,
                                                       tl[0:64, 384:449],
                                                       OP.mult, OP.add)
                    s_cur = s_nxt

                    prev = {"tl": tl, "st_m": st_m, "at": at, "qw": qw, "i": i,
                            "s_before": s_before}
                    if n in (4, 8, 12):
                        nc.sync.dma_start(svlv[p, :, n - 4:n, :],
                                          acc[:, n - 4:n, :])

                # ---- epilogue: finish last tile ----
                ptl = prev["tl"]
                nc.tensor.matmul(ptl[0:65, 512:512 + prev["qw"]],
                                 vsm_ap(prev["i"]),
                                 prev["st_m"][:, 0:prev["qw"]],
                                 start=True, stop=True)
                pc = NB - 1
                nc.tensor.matmul(ptl[0:65, 768:896], prev["s_before"][:],
                                 sgq_sb[:, pc * 128:pc * 128 + 128],
                                 start=True, stop=False)
                nc.tensor.matmul(ptl[0:65, 768:896], vlin_ap(pc),
                                 prev["at"][:], start=False, stop=True)
                nc.vector.tensor_copy(acc[:, NB - 1, :], ptl[0:65, 512:896])
                nc.sync.dma_start(svlv[p, :, 12:NB, :], acc[:, 12:NB, :])
                nc.sync.dma_start(tails[p], tails_acc[:])

    nc.compile()
    return nc


def host_prep(queries, keys, values, key_lengths_mask, blend):
    """Build per-core in_maps from full inputs."""
    q = np.ascontiguousarray(np.transpose(queries, (0, 2, 1, 3)))  # [N,H,L,E]
    k = np.ascontiguousarray(np.transpose(keys, (0, 2, 1, 3)))
    v = np.ascontiguousarray(np.transpose(values, (0, 2, 1, 3)))
    q = q.reshape(N * H, L, E).astype(np.float32)
    k = k.reshape(N * H, L, E).astype(np.float32)
    v = v.reshape(N * H, L, E).astype(np.float32)
    klm = np.asarray(key_lengths_mask, np.float32)  # [N, L]

    ii = np.arange(128)[:, None]
    cc = np.arange(256)[None, :]
    m01 = ((cc - ii >= 0) & (cc - ii <= 128)).astype(np.float32)

    in_maps = []
    for core in range(NCORES):
        qkts, sgs, vvkns = [], [], []
        for p in range(PAIRS_PER_CORE):
            g = core * PAIRS_PER_CORE + p
            n = g // H
            qg, kg, vg = q[g], k[g], v[g]          # [L, E]
            kl = klm[n]                             # [L]
            i01 = (kl > 0).astype(np.float32)

            qkt_p = np.empty((65, 2, L), np.float32)
            qkt_p[0:64, 0] = kg.T
            qkt_p[64, 0] = -1e9 * (1.0 - i01)
            qkt_p[0:64, 1] = qg.T
            qkt_p[64, 1] = 1.0

            phiq = np.tanh(qg) + 1.0
            phik = np.tanh(kg) + 1.0
            sg_p = np.empty((64, 2, L), np.float32)
            sg_p[:, 0] = phik.T
            sg_p[:, 1] = phiq.T

            vsm_full = np.empty((L, 65), np.float32)
            vsm_full[:, 0:64] = vg * i01[:, None]
            vsm_full[:, 64] = i01
            vsm_p = vsm_full.reshape(NB, 128, 65).transpose(1, 0, 2)  # [128,NB,65]
            vlin_full = np.empty((L, 65), np.float32)
            vlin_full[:, 0:64] = vg * kl[:, None]
            vlin_full[:, 64] = kl
            vlin_p = vlin_full.reshape(NB, 128, 65).transpose(1, 0, 2)
            sgkn_p = phik.reshape(NB, 128, 64).transpose(1, 0, 2).reshape(128, NB * 64)
            # columns: [vlin0 | vsm 0..15 | vlin 1..15 | sgkn]
            vvkn_p = np.concatenate(
                [vlin_p[:, 0], vsm_p.reshape(128, NB * 65),
                 vlin_p[:, 1:].reshape(128, (NB - 1) * 65), sgkn_p], axis=1)

            qkts.append(qkt_p.astype(ml_dtypes.bfloat16))
            sgs.append(sg_p.astype(ml_dtypes.bfloat16))
            vvkns.append(vvkn_p.astype(ml_dtypes.bfloat16))

        in_maps.append({
            "qkt": np.ascontiguousarray(np.stack(qkts)),
            "sg": np.ascontiguousarray(np.stack(sgs)),
            "vvkn": np.ascontiguousarray(np.stack(vvkns)),
            "m01d": np.ascontiguousarray(m01.astype(ml_dtypes.bfloat16)),
        })
    return in_maps


def assemble(results, blend):
    """Normalize + blend on host from raw numerators/denominators."""
    b = float(np.asarray(blend).reshape(-1)[0])
    full = np.empty((N, H, L, E), np.float32)
    for core in range(NCORES):
        r = results[core]
        svlv = np.asarray(r["svlv"], dtype=np.float32)   # [P, 65, NB, 384]
        tails = np.asarray(r["tails"])                   # [P, 128, NB]
        for p in range(PAIRS_PER_CORE):
            g = core * PAIRS_PER_CORE + p
            n, h = g // H, g % H
            # iteration n processed softmax block 15-n, linear chunk n
            sv = svlv[p, :, ::-1, 0:256]    # [65, block, 256] (block ascending)
            lv = svlv[p, :, :, 256:384]     # [65, chunk, 128]
            den = tails[p].T + sv[64, :, 0:128]          # [NB, 128]
            num = sv[0:64, :, 0:128].copy()              # [64, NB, 128]
            num[:, 1:, :] += sv[0:64, 0:NB - 1, 128:256]
            lvn = lv[0:64]                               # [64, NB, 128]
            lvd = lv[64]                                 # [NB, 128]
            o = (b * num / den[None] +
                 (1.0 - b) * lvn / (lvd[None] + EPS))    # [64, NB, 128]
            full[n, h] = o.transpose(1, 2, 0).reshape(L, E)
    return np.ascontiguousarray(np.transpose(full, (0, 2, 1, 3)))


def kernel(queries, keys, values, key_lengths_mask, blend, _trace=False):
    if "nc" not in _cached:
        _cached["nc"] = build_nc()
    nc = _cached["nc"]
    in_maps = host_prep(queries, keys, values, key_lengths_mask, blend)
    res = bass_utils.run_bass_kernel_spmd(nc, in_maps, core_ids=list(range(NCORES)),
                                          trace=_trace)
    _cached["last_results"] = res
    return assemble(res.results, blend)


# revision 17
# speedup vs baseline: 1.0261x; 1.0261x over previous
"""Trainium2 Bass kernel for nn_CausalRankKAttention.

Blend of banded-softmax attention and cumsum linear attention, per (n,h) pair.
16 pairs sharded over 8 NeuronCores (2 pairs/core), no cross-core comm.

Design (v6):
  - feature map phi(x)=tanh(x)+1 on HOST; only the exp table ever loads on ACT.
  - all matmuls bf16 (PE streams ~1 col/cycle at 1.2GHz regardless of dtype;
    fp8 DoubleRow measured no faster).
  - per block lb (ascending, = linear chunk lb):
      tail: forward scores q-block lb vs s-blocks [0, lb) -> wide psum
        [128, lb*128], one ACT exp with fused accum_out -> denominator tail.
      band: transposed tile st[k=lb, q in lb..lb+1] -> exp -> mask.
      linear: transposed scores -> causal mask -> kn state delta.
  - key trick: with a binary key mask, vsm == vlin == [v, klm]; the band "mv"
    matmul and the linear "atv" matmul share their stationary tensor, so one
    384-col matmul computes both (rhs = [st_m | at] written side by side by
    DVE), with the inter matmul accumulating into the last 128 cols of the
    same psum group. 5 small matmuls per iteration instead of 7.
  - dependent matmuls trail one iteration; band+linear share one [128, 1024]
    psum tile per iteration, drained by a single [65, 384] CAST.
  - outputs are RAW numerators/denominators; normalize + blend on host.
"""

import numpy as np
import ml_dtypes

import concourse.bass as bass
import concourse.bacc as bacc
import concourse.mybir as mybir
import concourse.tile as tile
from concourse import bass_utils

F32 = mybir.dt.float32
BF16 = mybir.dt.bfloat16
AF = mybir.ActivationFunctionType
OP = mybir.AluOpType

N, L, H, E = 2, 2048, 8, 64
NB = L // 128            # 16 blocks/chunks of 128
TEMP = float(1.0 / np.sqrt(E))
EPS = 1e-6
PAIRS_PER_CORE = 2
NCORES = 8

_cached = {}


def build_nc():
    nc = bacc.Bacc("TRN2", target_bir_lowering=False, debug=False,
                   num_devices=NCORES)
    P = PAIRS_PER_CORE
    # ---- dram tensors (per core) ----
    # qkt[p, :, 0] = kt (k^T + gate ext row), [p, :, 1] = qt (q^T + ones row)
    qkt = nc.dram_tensor("qkt", [P, 65, 2, L], BF16, kind="ExternalInput")
    sg = nc.dram_tensor("sg", [P, 64, 2, L], BF16, kind="ExternalInput")
    # vvkn: [vv (NB*65) | sgkn (NB*64)]
    vvkn = nc.dram_tensor("vvkn", [P, 128, NB * 65 + NB * 64], BF16,
                          kind="ExternalInput")
    m01d = nc.dram_tensor("m01d", [128, 256], BF16, kind="ExternalInput")
    svlv = nc.dram_tensor("svlv", [P, 65, NB, 384], BF16, kind="ExternalOutput")
    tails = nc.dram_tensor("tails", [P, 128, NB], F32, kind="ExternalOutput")

    with tile.TileContext(nc) as tc:
        with (
            tc.tile_pool(name="const", bufs=1) as constp,
            tc.tile_pool(name="io", bufs=2) as iop,
            tc.tile_pool(name="acc", bufs=2) as accp,
            tc.tile_pool(name="work", bufs=2) as workp,
            tc.tile_pool(name="sp", bufs=3) as sp,
            tc.tile_pool(name="tailp", bufs=1, space="PSUM") as tailp,
            tc.tile_pool(name="blp", bufs=2, space="PSUM") as blp,
        ):
            m01_sb = constp.tile([128, 256], BF16, tag="m01")
            nc.sync.dma_start(m01_sb[:], m01d[:])

            for p in range(P):
                qkt_sb = iop.tile([65, 2, L], BF16, tag="qkt")
                sg_sb = iop.tile([64, 2, L], BF16, tag="sg")
                vvkn_sb = iop.tile([128, NB * 65 + NB * 64], BF16, tag="vvkn")
                nc.sync.dma_start(qkt_sb[:, :, 0:1024], qkt[p, :, :, 0:1024])
                nc.sync.dma_start(sg_sb[:, :, 0:1024], sg[p, :, :, 0:1024])
                nc.sync.dma_start(vvkn_sb[:], vvkn[p])
                nc.sync.dma_start(qkt_sb[:, :, 1024:2048], qkt[p, :, :, 1024:2048])
                nc.sync.dma_start(sg_sb[:, :, 1024:2048], sg[p, :, :, 1024:2048])
                kt_sb = qkt_sb[:, 0, :]       # [65, L]
                qt_sb = qkt_sb[:, 1, :]
                sgk_sb = sg_sb[:, 0, :]
                sgq_sb = sg_sb[:, 1, :]

                def vv_ap(i):
                    return vvkn_sb[:, i * 65:(i + 1) * 65]

                def sgkn_ap(c):
                    return vvkn_sb[:, NB * 65 + c * 64:NB * 65 + (c + 1) * 64]

                acc = accp.tile([65, NB, 384], BF16, tag="acc")
                tails_acc = accp.tile([128, NB], F32, tag="tails")
                nc.gpsimd.memset(tails_acc[:], 0.0)

                s_cur = None     # state after chunk n (bf16 [64, 65])
                prev = None
                for n in range(NB):
                    qw = 256 if n < NB - 1 else 128
                    c0, c1 = n * 128, (n + 1) * 128

                    # ---- PE: band scores (transposed) ----
                    tl = blp.tile([128, 1024], F32, tag="tl")
                    nc.tensor.matmul(tl[:, 0:qw], kt_sb[:, c0:c1],
                                     qt_sb[:, c0:c0 + qw],
                                     start=True, stop=True)
                    # ---- PE: linear qk scores (transposed) ----
                    nc.tensor.matmul(tl[:, 256:384], sgk_sb[:, c0:c1],
                                     sgq_sb[:, c0:c1], start=True, stop=True)
                    # ---- PE: finish previous tile: merged mv+atv, inter ----
                    if prev is not None:
                        ptl = prev["tl"]
                        pn = n - 1
                        nc.tensor.matmul(ptl[0:65, 512:896], vv_ap(pn),
                                         prev["stat"][:], start=True,
                                         stop=(pn == 0), skip_group_check=True)
                        if pn > 0:
                            nc.tensor.matmul(ptl[0:65, 768:896],
                                             prev["s_before"][:],
                                             sgq_sb[:, pn * 128:pn * 128 + 128],
                                             start=False, stop=True,
                                             skip_group_check=True)
                        nc.vector.tensor_copy(acc[:, pn, :], ptl[0:65, 512:896])
                    # ---- PE: kn (state delta for chunk n) ----
                    nc.tensor.matmul(tl[0:64, 384:449], sgkn_ap(n),
                                     vv_ap(n), start=True, stop=True)
                    # ---- PE: tail for block n (last; WAR on previous exp
                    # overlaps the matmuls above) ----
                    if n >= 1:
                        w = n * 128
                        tp = tailp.tile([128, 2048], F32, tag="tp")
                        for off in range(0, w, 512):
                            n_ = min(512, w - off)
                            nc.tensor.matmul(tp[:, off:off + n_],
                                             qt_sb[:, c0:c1],
                                             kt_sb[:, off:off + n_],
                                             start=True, stop=True)

                    # ---- ACT: band exp, then tail exp (accum -> tails) ----
                    st_e = workp.tile([128, 256], BF16, tag="st_e")
                    nc.scalar.activation(st_e[:, 0:qw], tl[:, 0:qw], AF.Exp,
                                         scale=TEMP)
                    if n >= 1:
                        scrap = workp.tile([128, 1920], BF16, tag="scrap")
                        nc.scalar.activation(scrap[:, 0:w], tp[:, 0:w], AF.Exp,
                                             scale=TEMP,
                                             accum_out=tails_acc[:, n:n + 1])
                    # ---- DVE: masks into the shared rhs tile; state update ----
                    stat = workp.tile([128, 384], BF16, tag="stat")
                    nc.vector.tensor_tensor(stat[:, 256:384], tl[:, 256:384],
                                            m01_sb[:, 0:128], OP.mult)
                    nc.vector.tensor_tensor(stat[:, 0:qw], st_e[:, 0:qw],
                                            m01_sb[:, 0:qw], OP.mult)
                    if qw < 256:
                        nc.vector.memset(stat[:, 128:256], 0.0)
                    s_before = s_cur
                    s_nxt = sp.tile([64, 65], BF16, tag="s")
                    if n == 0:
                        nc.vector.tensor_copy(s_nxt[:], tl[0:64, 384:449])
                    else:
                        nc.vector.scalar_tensor_tensor(s_nxt[:], s_cur[:], 1.0,
                                                       tl[0:64, 384:449],
                                                       OP.mult, OP.add)
                    s_cur = s_nxt

                    prev = {"tl": tl, "stat": stat, "s_before": s_before}
                    if n in (5, 9, 13):
                        nc.sync.dma_start(svlv[p, :, n - 5:n - 1, :],
                                          acc[:, n - 5:n - 1, :])

                # ---- epilogue: finish last tile (block/chunk 15) ----
                ptl = prev["tl"]
                pn = NB - 1
                nc.tensor.matmul(ptl[0:65, 512:896], vv_ap(pn), prev["stat"][:],
                                 start=True, stop=False, skip_group_check=True)
                nc.tensor.matmul(ptl[0:65, 768:896], prev["s_before"][:],
                                 sgq_sb[:, pn * 128:pn * 128 + 128],
                                 start=False, stop=True, skip_group_check=True)
                nc.vector.tensor_copy(acc[:, pn, :], ptl[0:65, 512:896])
                # final output DMAs on the ACT queue: keeps the sync queue free
                # so the next pair's input DMAs prefetch during this pair
                nc.scalar.dma_start(svlv[p, :, 12:NB, :], acc[:, 12:NB, :])
                nc.scalar.dma_start(tails[p], tails_acc[:])

    nc.compile()
    return nc


def host_prep(queries, keys, values, key_lengths_mask, blend):
    """Build per-core in_maps from full inputs."""
    q = np.ascontiguousarray(np.transpose(queries, (0, 2, 1, 3)))  # [N,H,L,E]
    k = np.ascontiguousarray(np.transpose(keys, (0, 2, 1, 3)))
    v = np.ascontiguousarray(np.transpose(values, (0, 2, 1, 3)))
    q = q.reshape(N * H, L, E).astype(np.float32)
    k = k.reshape(N * H, L, E).astype(np.float32)
    v = v.reshape(N * H, L, E).astype(np.float32)
    klm = np.asarray(key_lengths_mask, np.float32)  # [N, L]

    ii = np.arange(128)[:, None]
    cc = np.arange(256)[None, :]
    m01 = ((cc - ii >= 0) & (cc - ii <= 128)).astype(np.float32)

    in_maps = []
    for core in range(NCORES):
        qkts, sgs, vvkns = [], [], []
        for p in range(PAIRS_PER_CORE):
            g = core * PAIRS_PER_CORE + p
            n = g // H
            qg, kg, vg = q[g], k[g], v[g]          # [L, E]
            kl = klm[n]                             # [L]
            i01 = (kl > 0).astype(np.float32)

            qkt_p = np.empty((65, 2, L), np.float32)
            qkt_p[0:64, 0] = kg.T
            qkt_p[64, 0] = -1e9 * (1.0 - i01)
            qkt_p[0:64, 1] = qg.T
            qkt_p[64, 1] = 1.0

            phiq = np.tanh(qg) + 1.0
            phik = np.tanh(kg) + 1.0
            sg_p = np.empty((64, 2, L), np.float32)
            sg_p[:, 0] = phik.T
            sg_p[:, 1] = phiq.T

            vv_full = np.empty((L, 65), np.float32)
            vv_full[:, 0:64] = vg * kl[:, None]
            vv_full[:, 64] = kl
            vv_p = vv_full.reshape(NB, 128, 65).transpose(1, 0, 2)
            sgkn_p = phik.reshape(NB, 128, 64).transpose(1, 0, 2).reshape(128, NB * 64)
            vvkn_p = np.concatenate([vv_p.reshape(128, NB * 65), sgkn_p], axis=1)

            qkts.append(qkt_p.astype(ml_dtypes.bfloat16))
            sgs.append(sg_p.astype(ml_dtypes.bfloat16))
            vvkns.append(vvkn_p.astype(ml_dtypes.bfloat16))

        in_maps.append({
            "qkt": np.ascontiguousarray(np.stack(qkts)),
            "sg": np.ascontiguousarray(np.stack(sgs)),
            "vvkn": np.ascontiguousarray(np.stack(vvkns)),
            "m01d": np.ascontiguousarray(m01.astype(ml_dtypes.bfloat16)),
        })
    return in_maps


def assemble(results, blend):
    """Normalize + blend on host from raw numerators/denominators."""
    b = float(np.asarray(blend).reshape(-1)[0])
    full = np.empty((N, H, L, E), np.float32)
    for core in range(NCORES):
        r = results[core]
        svlv = np.asarray(r["svlv"], dtype=np.float32)   # [P, 65, NB, 384]
        tails = np.asarray(r["tails"])                   # [P, 128, NB]
        for p in range(PAIRS_PER_CORE):
            g = core * PAIRS_PER_CORE + p
            n, h = g // H, g % H
            sv = svlv[p, :, :, 0:256]       # [65, block, 256]
            lv = svlv[p, :, :, 256:384]     # [65, chunk, 128]
            den = tails[p].T + sv[64, :, 0:128]          # [NB, 128]
            num = sv[0:64, :, 0:128].copy()              # [64, NB, 128]
            num[:, 1:, :] += sv[0:64, 0:NB - 1, 128:256]
            lvn = lv[0:64]                               # [64, NB, 128]
            lvd = lv[64]                                 # [NB, 128]
            o = (b * num / den[None] +
                 (1.0 - b) * lvn / (lvd[None] + EPS))    # [64, NB, 128]
            full[n, h] = o.transpose(1, 2, 0).reshape(L, E)
    return np.ascontiguousarray(np.transpose(full, (0, 2, 1, 3)))


def kernel(queries, keys, values, key_lengths_mask, blend, _trace=False):
    if "nc" not in _cached:
        _cached["nc"] = build_nc()
    nc = _cached["nc"]
    in_maps = host_prep(queries, keys, values, key_lengths_mask, blend)
    res = bass_utils.run_bass_kernel_spmd(nc, in_maps, core_ids=list(range(NCORES)),
                                          trace=_trace)
    _cached["last_results"] = res
    return assemble(res.results, blend)


# revision 20
# speedup vs baseline: 1.0592x; 1.0322x over previous
"""Trainium2 Bass kernel for nn_CausalRankKAttention.

Blend of banded-softmax attention and cumsum linear attention, per (n,h) pair.
16 pairs sharded over 8 NeuronCores (2 pairs/core), no cross-core comm.

Design (v6):
  - feature map phi(x)=tanh(x)+1 on HOST; only the exp table ever loads on ACT.
  - all matmuls bf16 (PE streams ~1 col/cycle at 1.2GHz regardless of dtype;
    fp8 DoubleRow measured no faster).
  - per block lb (ascending, = linear chunk lb):
      tail: forward scores q-block lb vs s-blocks [0, lb) -> wide psum
        [128, lb*128], one ACT exp with fused accum_out -> denominator tail.
      band: transposed tile st[k=lb, q in lb..lb+1] -> exp -> mask.
      linear: transposed scores -> causal mask -> kn state delta.
  - key trick: with a binary key mask, vsm == vlin == [v, klm]; the band "mv"
    matmul and the linear "atv" matmul share their stationary tensor, so one
    384-col matmul computes both (rhs = [st_m | at] written side by side by
    DVE), with the inter matmul accumulating into the last 128 cols of the
    same psum group. 5 small matmuls per iteration instead of 7.
  - dependent matmuls trail one iteration; band+linear share one [128, 1024]
    psum tile per iteration, drained by a single [65, 384] CAST.
  - outputs are RAW numerators/denominators; normalize + blend on host.
"""

import numpy as np
import ml_dtypes

import concourse.bass as bass
import concourse.bacc as bacc
import concourse.mybir as mybir
import concourse.tile as tile
from concourse import bass_utils

F32 = mybir.dt.float32
BF16 = mybir.dt.bfloat16
AF = mybir.ActivationFunctionType
OP = mybir.AluOpType

N, L, H, E = 2, 2048, 8, 64
NB = L // 128            # 16 blocks/chunks of 128
TEMP = float(1.0 / np.sqrt(E))
EPS = 1e-6
PAIRS_PER_CORE = 2
NCORES = 8

_cached = {}


def build_nc():
    nc = bacc.Bacc("TRN2", target_bir_lowering=False, debug=False,
                   num_devices=NCORES)
    P = PAIRS_PER_CORE
    # ---- dram tensors (per core) ----
    # qkt[p, :, 0] = kt (k^T + gate ext row), [p, :, 1] = qt (q^T + ones row)
    qkt = nc.dram_tensor("qkt", [P, 65, 2, L], BF16, kind="ExternalInput")
    sg = nc.dram_tensor("sg", [P, 64, 2, L], BF16, kind="ExternalInput")
    # vvkn: [vv (NB*65) | sgkn (NB*64)]
    vvkn = nc.dram_tensor("vvkn", [P, 128, NB * 65 + NB * 64], BF16,
                          kind="ExternalInput")
    m01d = nc.dram_tensor("m01d", [128, 256], BF16, kind="ExternalInput")
    svlv = nc.dram_tensor("svlv", [P, 65, NB, 384], BF16, kind="ExternalOutput")
    tails = nc.dram_tensor("tails", [P, 128, NB], F32, kind="ExternalOutput")

    with tile.TileContext(nc) as tc:
        with (
            tc.tile_pool(name="const", bufs=1) as constp,
            tc.tile_pool(name="io", bufs=2) as iop,
            tc.tile_pool(name="acc", bufs=2) as accp,
            tc.tile_pool(name="work", bufs=2) as workp,
            tc.tile_pool(name="sp", bufs=3) as sp,
            tc.tile_pool(name="tailp", bufs=1, space="PSUM") as tailp,
            tc.tile_pool(name="scp", bufs=2, space="PSUM") as scp,
            tc.tile_pool(name="otp", bufs=2, space="PSUM") as otp,
        ):
            m01_sb = constp.tile([128, 256], BF16, tag="m01")
            nc.sync.dma_start(m01_sb[:], m01d[:])

            for p in range(P):
                qkt_sb = iop.tile([65, 2, L], BF16, tag="qkt")
                sg_sb = iop.tile([64, 2, L], BF16, tag="sg")
                vvkn_sb = iop.tile([128, NB * 65 + NB * 64], BF16, tag="vvkn")
                nc.sync.dma_start(qkt_sb[:, :, 0:1024], qkt[p, :, :, 0:1024])
                nc.sync.dma_start(qkt_sb[:, :, 1024:2048], qkt[p, :, :, 1024:2048])
                nc.sync.dma_start(sg_sb[:, :, 0:1024], sg[p, :, :, 0:1024])
                nc.sync.dma_start(vvkn_sb[:], vvkn[p])
                nc.sync.dma_start(sg_sb[:, :, 1024:2048], sg[p, :, :, 1024:2048])
                kt_sb = qkt_sb[:, 0, :]       # [65, L]
                qt_sb = qkt_sb[:, 1, :]
                sgk_sb = sg_sb[:, 0, :]
                sgq_sb = sg_sb[:, 1, :]

                def vv_ap(i):
                    return vvkn_sb[:, i * 65:(i + 1) * 65]

                def sgkn_ap(c):
                    return vvkn_sb[:, NB * 65 + c * 64:NB * 65 + (c + 1) * 64]

                acc = accp.tile([65, NB, 384], BF16, tag="acc")
                tails_acc = accp.tile([128, NB], F32, tag="tails")
                nc.gpsimd.memset(tails_acc[:], 0.0)

                # ---- hoisted: tail for block 15 (heaviest) runs first ----
                w15 = (NB - 1) * 128
                tp = tailp.tile([128, 2048], F32, tag="tp")
                for off in range(0, w15, 512):
                    n_ = min(512, w15 - off)
                    nc.tensor.matmul(tp[:, off:off + n_],
                                     qt_sb[:, w15:w15 + 128],
                                     kt_sb[:, off:off + n_],
                                     start=True, stop=True)
                scrap = workp.tile([128, 1920], BF16, tag="scrap")
                nc.scalar.activation(scrap[:, 0:w15], tp[:, 0:w15], AF.Exp,
                                     scale=TEMP,
                                     accum_out=tails_acc[:, NB - 1:NB])

                s_cur = None     # state after chunk n (bf16 [64, 65])
                prev = None
                for n in range(NB):
                    qw = 256 if n < NB - 1 else 128
                    c0, c1 = n * 128, (n + 1) * 128

                    # ---- PE: band scores (transposed) + linear qk + kn ----
                    sc = scp.tile([128, 512], F32, tag="sc")
                    nc.tensor.matmul(sc[:, 0:qw], kt_sb[:, c0:c1],
                                     qt_sb[:, c0:c0 + qw],
                                     start=True, stop=True)
                    nc.tensor.matmul(sc[:, 256:384], sgk_sb[:, c0:c1],
                                     sgq_sb[:, c0:c1], start=True, stop=True)
                    # ---- PE: finish previous block: merged mv+atv, inter ----
                    if prev is not None:
                        pot = prev["ot"]
                        pn = n - 1
                        nc.tensor.matmul(pot[:], vv_ap(pn),
                                         prev["stat"][:], start=True,
                                         stop=(pn == 0), skip_group_check=True)
                        if pn > 0:
                            nc.tensor.matmul(pot[:, 256:384],
                                             prev["s_before"][:],
                                             sgq_sb[:, pn * 128:pn * 128 + 128],
                                             start=False, stop=True,
                                             skip_group_check=True)
                        nc.vector.tensor_copy(acc[:, pn, :], pot[:])
                    nc.tensor.matmul(sc[0:64, 384:449], sgkn_ap(n),
                                     vv_ap(n), start=True, stop=True)
                    # ---- PE: tail for block n (last; WAR on previous exp
                    # overlaps the matmuls above) ----
                    if 1 <= n < NB - 1:
                        w = n * 128
                        tp = tailp.tile([128, 2048], F32, tag="tp")
                        for off in range(0, w, 512):
                            n_ = min(512, w - off)
                            nc.tensor.matmul(tp[:, off:off + n_],
                                             qt_sb[:, c0:c1],
                                             kt_sb[:, off:off + n_],
                                             start=True, stop=True)

                    # ---- ACT: band exp, then tail exp (accum -> tails) ----
                    st_e = workp.tile([128, 256], BF16, tag="st_e")
                    nc.scalar.activation(st_e[:, 0:qw], sc[:, 0:qw], AF.Exp,
                                         scale=TEMP)
                    if 1 <= n < NB - 1:
                        scrap = workp.tile([128, 1920], BF16, tag="scrap")
                        nc.scalar.activation(scrap[:, 0:w], tp[:, 0:w], AF.Exp,
                                             scale=TEMP,
                                             accum_out=tails_acc[:, n:n + 1])
                    # ---- DVE: masks into the shared rhs tile; state update ----
                    stat = workp.tile([128, 384], BF16, tag="stat")
                    nc.vector.tensor_tensor(stat[:, 256:384], sc[:, 256:384],
                                            m01_sb[:, 0:128], OP.mult)
                    nc.vector.tensor_tensor(stat[:, 0:qw], st_e[:, 0:qw],
                                            m01_sb[:, 0:qw], OP.mult)
                    if qw < 256:
                        nc.vector.memset(stat[:, 128:256], 0.0)
                    s_before = s_cur
                    s_nxt = sp.tile([64, 65], BF16, tag="s")
                    if n == 0:
                        nc.vector.tensor_copy(s_nxt[:], sc[0:64, 384:449])
                    else:
                        nc.vector.scalar_tensor_tensor(s_nxt[:], s_cur[:], 1.0,
                                                       sc[0:64, 384:449],
                                                       OP.mult, OP.add)
                    s_cur = s_nxt

                    ot = otp.tile([65, 384], F32, tag="ot")
                    prev = {"ot": ot, "stat": stat, "s_before": s_before}
                    if n in (5, 9, 13):
                        nc.sync.dma_start(svlv[p, :, n - 5:n - 1, :],
                                          acc[:, n - 5:n - 1, :])

                # ---- epilogue: finish last block (block/chunk 15) ----
                pot = prev["ot"]
                pn = NB - 1
                nc.tensor.matmul(pot[:], vv_ap(pn), prev["stat"][:],
                                 start=True, stop=False, skip_group_check=True)
                nc.tensor.matmul(pot[:, 256:384], prev["s_before"][:],
                                 sgq_sb[:, pn * 128:pn * 128 + 128],
                                 start=False, stop=True, skip_group_check=True)
                nc.vector.tensor_copy(acc[:, pn, :], pot[:])
                # final output DMAs on the ACT queue: keeps the sync queue free
                # so the next pair's input DMAs prefetch during this pair
                nc.scalar.dma_start(svlv[p, :, 12:NB, :], acc[:, 12:NB, :])
                nc.scalar.dma_start(tails[p], tails_acc[:])

    nc.compile()
    return nc


def host_prep(queries, keys, values, key_lengths_mask, blend):
    """Build per-core in_maps from full inputs."""
    q = np.ascontiguousarray(np.transpose(queries, (0, 2, 1, 3)))  # [N,H,L,E]
    k = np.ascontiguousarray(np.transpose(keys, (0, 2, 1, 3)))
    v = np.ascontiguousarray(np.transpose(values, (0, 2, 1, 3)))
    q = q.reshape(N * H, L, E).astype(np.float32)
    k = k.reshape(N * H, L, E).astype(np.float32)
    v = v.reshape(N * H, L, E).astype(np.float32)
    klm = np.asarray(key_lengths_mask, np.float32)  # [N, L]

    ii = np.arange(128)[:, None]
    cc = np.arange(256)[None, :]
    m01 = ((cc - ii >= 0) & (cc - ii <= 128)).astype(np.float32)

    in_maps = []
    for core in range(NCORES):
        qkts, sgs, vvkns = [], [], []
        for p in range(PAIRS_PER_CORE):
            g = core * PAIRS_PER_CORE + p
            n = g // H
            qg, kg, vg = q[g], k[g], v[g]          # [L, E]
            kl = klm[n]                             # [L]
            i01 = (kl > 0).astype(np.float32)

            qkt_p = np.empty((65, 2, L), np.float32)
            qkt_p[0:64, 0] = kg.T
            qkt_p[64, 0] = -1e9 * (1.0 - i01)
            qkt_p[0:64, 1] = qg.T
            qkt_p[64, 1] = 1.0

            phiq = np.tanh(qg) + 1.0
            phik = np.tanh(kg) + 1.0
            sg_p = np.empty((64, 2, L), np.float32)
            sg_p[:, 0] = phik.T
            sg_p[:, 1] = phiq.T

            vv_full = np.empty((L, 65), np.float32)
            vv_full[:, 0:64] = vg * kl[:, None]
            vv_full[:, 64] = kl
            vv_p = vv_full.reshape(NB, 128, 65).transpose(1, 0, 2)
            sgkn_p = phik.reshape(NB, 128, 64).transpose(1, 0, 2).reshape(128, NB * 64)
            vvkn_p = np.concatenate([vv_p.reshape(128, NB * 65), sgkn_p], axis=1)

            qkts.append(qkt_p.astype(ml_dtypes.bfloat16))
            sgs.append(sg_p.astype(ml_dtypes.bfloat16))
            vvkns.append(vvkn_p.astype(ml_dtypes.bfloat16))

        in_maps.append({
            "qkt": np.ascontiguousarray(np.stack(qkts)),
            "sg": np.ascontiguousarray(np.stack(sgs)),
            "vvkn": np.ascontiguousarray(np.stack(vvkns)),
            "m01d": np.ascontiguousarray(m01.astype(ml_dtypes.bfloat16)),
        })
    return in_maps


def assemble(results, blend):
    """Normalize + blend on host from raw numerators/denominators."""
    b = float(np.asarray(blend).reshape(-1)[0])
    full = np.empty((N, H, L, E), np.float32)
    for core in range(NCORES):
        r = results[core]
        svlv = np.asarray(r["svlv"], dtype=np.float32)   # [P, 65, NB, 384]
        tails = np.asarray(r["tails"])                   # [P, 128, NB]
        for p in range(PAIRS_PER_CORE):
            g = core * PAIRS_PER_CORE + p
            n, h = g // H, g % H
            sv = svlv[p, :, :, 0:256]       # [65, block, 256]
            lv = svlv[p, :, :, 256:384]     # [65, chunk, 128]
            den = tails[p].T + sv[64, :, 0:128]          # [NB, 128]
            num = sv[0:64, :, 0:128].copy()              # [64, NB, 128]
            num[:, 1:, :] += sv[0:64, 0:NB - 1, 128:256]
            lvn = lv[0:64]                               # [64, NB, 128]
            lvd = lv[64]                                 # [NB, 128]
            o = (b * num / den[None] +
                 (1.0 - b) * lvn / (lvd[None] + EPS))    # [64, NB, 128]
            full[n, h] = o.transpose(1, 2, 0).reshape(L, E)
    return np.ascontiguousarray(np.transpose(full, (0, 2, 1, 3)))


def kernel(queries, keys, values, key_lengths_mask, blend, _trace=False):
    if "nc" not in _cached:
        _cached["nc"] = build_nc()
    nc = _cached["nc"]
    in_maps = host_prep(queries, keys, values, key_lengths_mask, blend)
    res = bass_utils.run_bass_kernel_spmd(nc, in_maps, core_ids=list(range(NCORES)),
                                          trace=_trace)
    _cached["last_results"] = res
    return assemble(res.results, blend)
